# revision 8
# baseline (speedup 1.0000x reference)
"""Benes butterfly network (12 layers, N=4096) on 8 Trainium2 NeuronCores.

Self-contained: takes full inputs, shards batch across 8 cores, runs a
Bass/Tile kernel per core, gathers the full output.

Math: reference layer k is a butterfly with span 2^k:
    h[:, j] <- A_k[j] * h[:, j] + B_k[j] * h[:, j ^ 2^k]
(A_k/B_k extracted from the sparse COO (values, idx_in, idx_out)).

V3 design (prior version spent ~28us of PE on diagonal matmuls whose
only purpose was transposing the output back to [batch, col]; since the
input is already transposed on the host, the output transpose moves to
the host gather too, and the device works column-major end to end):
  - Layers 0..8 composed on the host into dense 128x128 block matrices
    (mst, bf16) with diag(A9*A10*A11) folded in, so the later stt chain
    needs no trailing per-partition rescale:
      p1[t] = sum_{ji<4} M[t, t^ji] @ H0[t^ji]    (PE, N=512, fp32 psum)
  - Layers 9/10/11 as three scalar_tensor_tensor passes over bf16 SBUF
    tiles (per-partition scalar = per-column coefficient ratio):
      Y[t]   = p1[t] + s9[t]  * p1[t^4]    ( = A10*A11*H9[t] )
      Z[t]   = Y[t]  + s10[t] * Y[t^8]     ( = A11*H10[t] )
      H11[t] = Z[t]  + r11[t] * Z[t^16]    ( = H11[t] exactly )
    stt9/stt11 on DVE (bf16 2x mode), stt10 on GPSIMD.
  - Phase-1 psums evacuated to bf16 by ACT in fused [128,1024] copies.
  - Output written column-major (outT [N, BSH] bf16, contiguous 1KB
    lines); the host gather transposes + upcasts.
  - Jobs (residue, m-pair) flat-ordered to interleave residues 0/1
    during the xT load ramp; DMA issues paced by trigger-copies on the
    ACT queue; warmup matmuls lift the PE HAM throttle during the ramp.
"""
import os
import numpy as np
import ml_dtypes

N = 4096
BATCH = 4096
NLAYERS = 12
NCORES = 8
BSH = BATCH // NCORES      # 512 batch rows per core
T = N // 128               # 32 column tiles

_PROGRAM_CACHE = {}
LAST_EXEC_NS = None


def _extract_ab(values, idx_in, idx_out):
    """Per-layer butterfly coefficients A[k], B[k] (float64 [L, N])."""
    v = np.asarray(values, np.float64)
    ii = np.asarray(idx_in, np.int64)
    io = np.asarray(idx_out, np.int64)
    L, nnz = v.shape
    n = nnz // 2
    A = np.zeros((L, n))
    B = np.zeros((L, n))
    for k in range(L):
        s = 1 << k
        self_m = ii[k] == io[k]
        part_m = ii[k] == (io[k] ^ s)
        if not np.all(self_m | part_m):
            raise ValueError(f"layer {k}: unexpected sparse index structure")
        np.add.at(A[k], io[k][self_m], v[k][self_m])
        np.add.at(B[k], io[k][part_m], v[k][part_m])
    return A, B


def _clamp(a):
    return np.where(np.abs(a) < 1e-12, 1e-12, a)


def _host_precompute(values, idx_in, idx_out):
    A, B = _extract_ab(values, idx_in, idx_out)
    Ab = A.reshape(NLAYERS, T, 128)
    Bb = B.reshape(NLAYERS, T, 128)
    j = np.arange(128)

    # Dense composition of within-block layers 0..6, one 128x128 per tile.
    S = [np.eye(128) for _ in range(T)]
    for k in range(7):
        s = 1 << k
        for t in range(T):
            W = np.zeros((128, 128))
            W[j, j] = Ab[k, t]
            W[j, j ^ s] = Bb[k, t]
            S[t] = W @ S[t]
    # Cross-block layers 7, 8 (tile distances 1, 2): dict src_tile -> 128x128
    Sd = [{t: S[t]} for t in range(T)]
    for k in (7, 8):
        d = 1 << (k - 7)
        newS = []
        for t in range(T):
            out = {}
            for src, M in Sd[t].items():
                out[src] = Ab[k, t][:, None] * M
            for src, M in Sd[t ^ d].items():
                out[src] = out.get(src, 0) + Bb[k, t][:, None] * M
            newS.append(out)
        Sd = newS

    D = Ab[9] * Ab[10] * Ab[11]          # folded diagonal, [T, 128]

    # mst (bf16): column block for (r, m) at (r*8 + m)*512, 4 ji-blocks of
    # (diag(D[t]) @ Sd[t][t^ji]).T so matmul computes M @ H0.
    mst = np.zeros((128, T * 512), np.float32)
    for r in range(4):
        for m in range(8):
            t = 4 * m + r
            assert set(Sd[t].keys()) == {t, t ^ 1, t ^ 2, t ^ 3}
            base = (r * 8 + m) * 512
            for ji in range(4):
                M = D[t][:, None] * Sd[t][t ^ ji]
                mst[:, base + ji * 128: base + (ji + 1) * 128] = (
                    M.T.astype(np.float32)
                )

    # stt scalar tables, bf16 [128, 192], even columns only so every
    # per-partition scalar AP is 4-byte aligned (DVE 2x-mode requirement):
    # col 2t = s9, 64+2t = s10, 128+2t = r11
    stab = np.zeros((128, 192), np.float64)
    for t in range(T):
        stab[:, 2 * t] = Ab[10, t] * Ab[11, t] * Bb[9, t] / _clamp(D[t ^ 4])
        stab[:, 64 + 2 * t] = (
            Ab[11, t] * Bb[10, t] / _clamp(Ab[10, t ^ 8] * Ab[11, t ^ 8])
        )
        stab[:, 128 + 2 * t] = Bb[11, t] / _clamp(Ab[11, t ^ 16])

    return mst.astype(ml_dtypes.bfloat16), stab.astype(ml_dtypes.bfloat16)


# Flat job order: (residue, m-pair); residues 0/1 interleaved so the PE
# has runnable work while the tail of xT is still loading.
_JOBS = [
    (0, 0), (0, 1), (1, 0), (0, 2), (1, 1), (0, 3), (1, 2), (1, 3),
    (2, 0), (2, 1), (2, 2), (2, 3), (3, 0), (3, 1), (3, 2), (3, 3),
]


def _build_program():
    import concourse.mybir as mybir
    import concourse.tile as tile
    from concourse import bacc

    f32 = mybir.dt.float32
    bf16 = mybir.dt.bfloat16
    mult = mybir.AluOpType.mult
    add = mybir.AluOpType.add

    nc = bacc.Bacc("TRN2", target_bir_lowering=False, debug=False)
    xT_ap = nc.dram_tensor("xT", [N, BSH], bf16, kind="ExternalInput").ap()
    mst_ap = nc.dram_tensor("mst", [128, T * 512], bf16, kind="ExternalInput").ap()
    stab_ap = nc.dram_tensor("stab", [128, 192], bf16, kind="ExternalInput").ap()
    out_ap = nc.dram_tensor("outT", [N, BSH], bf16, kind="ExternalOutput").ap()

    with tile.TileContext(nc) as tc:
        with (
            tc.tile_pool(name="const", bufs=1) as constp,
            tc.tile_pool(name="h0", bufs=8) as h0p,
            tc.tile_pool(name="mstp", bufs=8) as mstp,
            tc.tile_pool(name="e2", bufs=3) as e2p,
            tc.tile_pool(name="y", bufs=16) as yp,
            tc.tile_pool(name="z", bufs=24) as zp,
            tc.tile_pool(name="h11", bufs=14) as h11p,
            tc.tile_pool(name="trg", bufs=2) as trgp,
            tc.tile_pool(name="ps", bufs=3, space="PSUM") as psp,
            tc.tile_pool(name="pw", bufs=1, space="PSUM") as pwp,
        ):
            stab = constp.tile([128, 192], bf16, name="stab")
            nc.sync.dma_start(stab[:], stab_ap[:])

            msth = {}

            def issue_mst(r, half, eng):
                tl = mstp.tile(
                    [128, 2048], bf16, tag="mst", name=f"mst_{r}{half}"
                )
                eng.dma_start(
                    tl[:],
                    mst_ap[:, r * 4096 + half * 2048:
                           r * 4096 + (half + 1) * 2048],
                )
                msth[(r, half)] = tl

            def mst_slice(r, m, ji):
                tl = msth[(r, m // 4)]
                mm = m % 4
                return tl[:, mm * 512 + ji * 128: mm * 512 + (ji + 1) * 128]

            H0 = {}

            def issue_h0(m, eng):
                tl = h0p.tile([128, 2048], bf16, tag="h0", name=f"h0_{m}")
                src = xT_ap[m * 512:(m + 1) * 512, :].rearrange(
                    "(lt p) b -> p lt b", lt=4, p=128
                )
                eng.dma_start(tl[:].rearrange("p (lt b) -> p lt b", lt=4), src)
                H0[m] = tl

            # upfront: exactly job 0's inputs (more would share round-robin
            # DMA bandwidth and delay job 0's start)
            issue_mst(0, 0, nc.sync)
            issue_h0(0, nc.sync)
            issue_h0(1, nc.sync)

            # Warmup matmuls (only need stab): keep the PE busy during the
            # xT ramp so HAM is at K=8/8 when real matmuls arrive.
            warm = pwp.tile([64, 64], f32, tag="warm", name="warm")
            for _ in range(12):
                nc.tensor.matmul(
                    warm[:], stab[:, 0:64], stab[:, 64:128],
                    start=True, stop=True,
                )

            # DMA pacing: trig copies on the ACT queue gate the next issues
            # on an earlier chunk's ARRIVAL (not on compute progress).
            def trig_then(gate_tile, thunks):
                trg = trgp.tile([128, 8], bf16, tag="trg", name="trg")
                nc.scalar.copy(trg[:], gate_tile[:, 0:8])
                for th in thunks:
                    th()

            trig_then(H0[0], [
                lambda: issue_h0(2, nc.scalar),
                lambda: issue_h0(3, nc.scalar),
                lambda: issue_mst(1, 0, nc.scalar),
            ])

            # issues attached after job i's evac (scalar queue, in order)
            sched = {
                0: [lambda: issue_h0(4, nc.scalar),
                    lambda: issue_h0(5, nc.scalar),
                    lambda: issue_mst(0, 1, nc.scalar)],
                1: [lambda: issue_h0(6, nc.scalar),
                    lambda: issue_h0(7, nc.scalar)],
                2: [lambda: issue_mst(1, 1, nc.scalar)],
                3: [lambda: issue_mst(2, 0, nc.scalar)],
                4: [lambda: issue_mst(2, 1, nc.scalar)],
                5: [lambda: issue_mst(3, 0, nc.scalar)],
                6: [lambda: issue_mst(3, 1, nc.scalar)],
            }

            E2 = {}      # (r, mp) -> [128, 1024] bf16 (tiles 4*(2mp)+r | 4*(2mp+1)+r)
            Y = {}
            Z = {}
            H11 = {}
            pend11 = []  # deferred stt11 work items (r, m)

            def Ecol(r, m):
                tl = E2[(r, m >> 1)]
                h = m & 1
                return tl[:, h * 512:(h + 1) * 512]

            def emit_stt11(r, m, eng):
                t = 4 * m + r
                H11[t] = h11p.tile([128, BSH], bf16, tag="h11", name=f"H11_{t}")
                eng.scalar_tensor_tensor(
                    H11[t][:], Z[4 * (m ^ 4) + r][:], stab[:, 128 + 2 * t:128 + 2 * t + 1],
                    Z[t][:], op0=mult, op1=add,
                )
                nc.sync.dma_start(out_ap[t * 128:(t + 1) * 128, :], H11[t][:])

            for idx, (r, mp) in enumerate(_JOBS):
                # ---- phase 1: 8 matmuls into a [128, 1024] psum pair
                P2 = psp.tile([128, 1024], f32, tag="ps", name=f"p2_{r}_{mp}")
                for h, m in enumerate((2 * mp, 2 * mp + 1)):
                    for ji in range(4):
                        nc.tensor.matmul(
                            P2[:, h * 512:(h + 1) * 512],
                            mst_slice(r, m, ji),
                            H0[m][:, (r ^ ji) * 512:((r ^ ji) + 1) * 512],
                            start=(ji == 0), stop=(ji == 3),
                        )
                # ---- fused ACT evacuation to bf16
                e2 = e2p.tile([128, 1024], bf16, tag="e2", name=f"e2_{r}_{mp}")
                nc.scalar.copy(e2[:], P2[:])
                E2[(r, mp)] = e2
                for th in sched.get(idx, []):
                    th()

                # ---- stt9 on DVE (partners t, t^4 are the two e2 halves)
                for h, m in enumerate((2 * mp, 2 * mp + 1)):
                    t = 4 * m + r
                    Y[t] = yp.tile([128, BSH], bf16, tag="y", name=f"Y_{t}")
                    nc.vector.scalar_tensor_tensor(
                        Y[t][:], Ecol(r, m ^ 1), stab[:, 2 * t:2 * t + 1],
                        Ecol(r, m), op0=mult, op1=add,
                    )

                # ---- drain up to 2 deferred stt11 (DVE) + their stores
                for _ in range(2):
                    if pend11:
                        pr, pm = pend11.pop(0)
                        emit_stt11(pr, pm, nc.vector)

                # ---- stt10 on GPSIMD per completed half (partners m, m^2)
                if mp in (1, 3):
                    for m in (range(0, 4) if mp == 1 else range(4, 8)):
                        t = 4 * m + r
                        Z[t] = zp.tile([128, BSH], bf16, tag="z", name=f"Z_{t}")
                        nc.vector.scalar_tensor_tensor(
                            Z[t][:], Y[4 * (m ^ 2) + r][:],
                            stab[:, 64 + 2 * t:64 + 2 * t + 1], Y[t][:],
                            op0=mult, op1=add,
                        )
                if mp == 3:
                    pend11.extend((r, m) for m in range(8))

            # tail: drain remaining stt11 on DVE
            for pr, pm in pend11:
                emit_stt11(pr, pm, nc.vector)

    nc.compile()
    return nc


def kernel(x, values, idx_in, idx_out):
    global LAST_EXEC_NS
    from concourse.bass_utils import run_bass_kernel_spmd

    x = np.asarray(x, np.float32)
    assert x.shape == (BATCH, N), x.shape
    mst, stab = _host_precompute(values, idx_in, idx_out)
    xT = np.ascontiguousarray(x.T.astype(ml_dtypes.bfloat16))

    if "prog" not in _PROGRAM_CACHE:
        _PROGRAM_CACHE["prog"] = _build_program()
    nc = _PROGRAM_CACHE["prog"]

    in_maps = [
        {
            "xT": np.ascontiguousarray(xT[:, i * BSH:(i + 1) * BSH]),
            "mst": mst,
            "stab": stab,
        }
        for i in range(NCORES)
    ]
    res = run_bass_kernel_spmd(nc, in_maps, core_ids=list(range(NCORES)))
    if os.environ.get("BENES_TRACE"):
        tres = run_bass_kernel_spmd(
            nc, in_maps, core_ids=list(range(NCORES)), trace=True
        )
        LAST_EXEC_NS = tres.exec_time_ns
        _PROGRAM_CACHE["profile_json"] = tres.profile_json
    out = np.empty((BATCH, N), np.float32)
    for i in range(NCORES):
        out[i * BSH:(i + 1) * BSH] = (
            np.asarray(res.results[i]["outT"]).T.astype(np.float32)
        )
    return out


# revision 11
# speedup vs baseline: 1.0794x; 1.0794x over previous
"""Benes butterfly network (12 layers, N=4096) on 8 Trainium2 NeuronCores.

Self-contained: takes full inputs, shards batch across 8 cores, runs a
Bass/Tile kernel per core, gathers the full output.

Math: reference layer k is a butterfly with span 2^k:
    h[:, j] <- A_k[j] * h[:, j] + B_k[j] * h[:, j ^ 2^k]
(A_k/B_k extracted from the sparse COO (values, idx_in, idx_out)).

V3 design (prior version spent ~28us of PE on diagonal matmuls whose
only purpose was transposing the output back to [batch, col]; since the
input is already transposed on the host, the output transpose moves to
the host gather too, and the device works column-major end to end):
  - Layers 0..8 composed on the host into dense 128x128 block matrices
    (mst, bf16) with diag(A9*A10*A11) folded in, so the later stt chain
    needs no trailing per-partition rescale:
      p1[t] = sum_{ji<4} M[t, t^ji] @ H0[t^ji]    (PE, N=512, fp32 psum)
  - Layers 9/10/11 as three scalar_tensor_tensor passes over bf16 SBUF
    tiles (per-partition scalar = per-column coefficient ratio):
      Y[t]   = p1[t] + s9[t]  * p1[t^4]    ( = A10*A11*H9[t] )
      Z[t]   = Y[t]  + s10[t] * Y[t^8]     ( = A11*H10[t] )
      H11[t] = Z[t]  + r11[t] * Z[t^16]    ( = H11[t] exactly )
    stt9/stt11 on DVE (bf16 2x mode), stt10 on GPSIMD.
  - Phase-1 psums evacuated to bf16 by ACT in fused [128,1024] copies.
  - Output written column-major (outT [N, BSH] bf16, contiguous 1KB
    lines); the host gather transposes + upcasts.
  - Jobs (residue, m-pair) flat-ordered to interleave residues 0/1
    during the xT load ramp; DMA issues paced by trigger-copies on the
    ACT queue; warmup matmuls lift the PE HAM throttle during the ramp.
"""
import os
import numpy as np
import ml_dtypes

N = 4096
BATCH = 4096
NLAYERS = 12
NCORES = 8
BSH = BATCH // NCORES      # 512 batch rows per core
T = N // 128               # 32 column tiles

_PROGRAM_CACHE = {}
LAST_EXEC_NS = None


def _extract_ab(values, idx_in, idx_out):
    """Per-layer butterfly coefficients A[k], B[k] (float64 [L, N])."""
    v = np.asarray(values, np.float64)
    ii = np.asarray(idx_in, np.int64)
    io = np.asarray(idx_out, np.int64)
    L, nnz = v.shape
    n = nnz // 2
    A = np.zeros((L, n))
    B = np.zeros((L, n))
    for k in range(L):
        s = 1 << k
        self_m = ii[k] == io[k]
        part_m = ii[k] == (io[k] ^ s)
        if not np.all(self_m | part_m):
            raise ValueError(f"layer {k}: unexpected sparse index structure")
        np.add.at(A[k], io[k][self_m], v[k][self_m])
        np.add.at(B[k], io[k][part_m], v[k][part_m])
    return A, B


def _clamp(a):
    return np.where(np.abs(a) < 1e-12, 1e-12, a)


def _host_precompute(values, idx_in, idx_out):
    A, B = _extract_ab(values, idx_in, idx_out)
    Ab = A.reshape(NLAYERS, T, 128)
    Bb = B.reshape(NLAYERS, T, 128)
    j = np.arange(128)

    # Dense composition of within-block layers 0..6, one 128x128 per tile.
    S = [np.eye(128) for _ in range(T)]
    for k in range(7):
        s = 1 << k
        for t in range(T):
            W = np.zeros((128, 128))
            W[j, j] = Ab[k, t]
            W[j, j ^ s] = Bb[k, t]
            S[t] = W @ S[t]
    # Cross-block layers 7, 8 (tile distances 1, 2): dict src_tile -> 128x128
    Sd = [{t: S[t]} for t in range(T)]
    for k in (7, 8):
        d = 1 << (k - 7)
        newS = []
        for t in range(T):
            out = {}
            for src, M in Sd[t].items():
                out[src] = Ab[k, t][:, None] * M
            for src, M in Sd[t ^ d].items():
                out[src] = out.get(src, 0) + Bb[k, t][:, None] * M
            newS.append(out)
        Sd = newS

    D = Ab[9] * Ab[10] * Ab[11]          # folded diagonal, [T, 128]

    # mst (bf16): column block for (r, m) at (r*8 + m)*512, 4 ji-blocks of
    # (diag(D[t]) @ Sd[t][t^ji]).T so matmul computes M @ H0.
    mst = np.zeros((128, T * 512), np.float32)
    for r in range(4):
        for m in range(8):
            t = 4 * m + r
            assert set(Sd[t].keys()) == {t, t ^ 1, t ^ 2, t ^ 3}
            base = (r * 8 + m) * 512
            for ji in range(4):
                M = D[t][:, None] * Sd[t][t ^ ji]
                mst[:, base + ji * 128: base + (ji + 1) * 128] = (
                    M.T.astype(np.float32)
                )

    # stt scalar tables, bf16 [128, 192], even columns only so every
    # per-partition scalar AP is 4-byte aligned (DVE 2x-mode requirement):
    # col 2t = s9, 64+2t = s10, 128+2t = r11
    stab = np.zeros((128, 192), np.float64)
    for t in range(T):
        stab[:, 2 * t] = Ab[10, t] * Ab[11, t] * Bb[9, t] / _clamp(D[t ^ 4])
        stab[:, 64 + 2 * t] = (
            Ab[11, t] * Bb[10, t] / _clamp(Ab[10, t ^ 8] * Ab[11, t ^ 8])
        )
        stab[:, 128 + 2 * t] = Bb[11, t] / _clamp(Ab[11, t ^ 16])

    stabf = np.zeros((128, 32), np.float64)
    for t in range(T):
        stabf[:, t] = stab[:, 64 + 2 * t]
    return (mst.astype(ml_dtypes.bfloat16), stab.astype(ml_dtypes.bfloat16),
            stabf.astype(np.float32))


# Flat job order: (residue, m-pair); residues 0/1 interleaved so the PE
# has runnable work while the tail of xT is still loading.
_JOBS = [
    (0, 0), (0, 1), (1, 0), (0, 2), (1, 1), (0, 3), (1, 2), (1, 3),
    (2, 0), (2, 1), (2, 2), (2, 3), (3, 0), (3, 1), (3, 2), (3, 3),
]


def _build_program():
    import concourse.mybir as mybir
    import concourse.tile as tile
    from concourse import bacc

    f32 = mybir.dt.float32
    bf16 = mybir.dt.bfloat16
    mult = mybir.AluOpType.mult
    add = mybir.AluOpType.add

    nc = bacc.Bacc("TRN2", target_bir_lowering=False, debug=False)
    xT_ap = nc.dram_tensor("xT", [N, BSH], bf16, kind="ExternalInput").ap()
    mst_ap = nc.dram_tensor("mst", [128, T * 512], bf16, kind="ExternalInput").ap()
    stab_ap = nc.dram_tensor("stab", [128, 192], bf16, kind="ExternalInput").ap()
    stabf_ap = nc.dram_tensor("stabf", [128, 32], f32, kind="ExternalInput").ap()
    out_ap = nc.dram_tensor("outT", [N, BSH], bf16, kind="ExternalOutput").ap()

    with tile.TileContext(nc) as tc:
        with (
            tc.tile_pool(name="const", bufs=1) as constp,
            tc.tile_pool(name="h0", bufs=8) as h0p,
            tc.tile_pool(name="mstp", bufs=8) as mstp,
            tc.tile_pool(name="e2", bufs=3) as e2p,
            tc.tile_pool(name="y", bufs=16) as yp,
            tc.tile_pool(name="z", bufs=24) as zp,
            tc.tile_pool(name="h11", bufs=14) as h11p,
            tc.tile_pool(name="trg", bufs=2) as trgp,
            tc.tile_pool(name="wk", bufs=6) as wkp,
            tc.tile_pool(name="ps", bufs=3, space="PSUM") as psp,
        ):
            stab = constp.tile([128, 192], bf16, name="stab")
            stabf = constp.tile([128, 32], f32, name="stabf")

            msth = {}

            def issue_mst(r, half, eng):
                tl = mstp.tile(
                    [128, 2048], bf16, tag="mst", name=f"mst_{r}{half}"
                )
                eng.dma_start(
                    tl[:],
                    mst_ap[:, r * 4096 + half * 2048:
                           r * 4096 + (half + 1) * 2048],
                )
                msth[(r, half)] = tl

            def mst_slice(r, m, ji):
                tl = msth[(r, m // 4)]
                mm = m % 4
                return tl[:, mm * 512 + ji * 128: mm * 512 + (ji + 1) * 128]

            H0 = {}

            def issue_h0(m, eng):
                tl = h0p.tile([128, 2048], bf16, tag="h0", name=f"h0_{m}")
                src = xT_ap[m * 512:(m + 1) * 512, :].rearrange(
                    "(lt p) b -> p lt b", lt=4, p=128
                )
                eng.dma_start(tl[:].rearrange("p (lt b) -> p lt b", lt=4), src)
                H0[m] = tl

            # upfront, in need-order on one HWDGE ring (FIFO per ring):
            # job 0's inputs first, scalar tables last (first use ~14us)
            issue_mst(0, 0, nc.sync)
            issue_h0(0, nc.sync)
            issue_h0(1, nc.sync)
            nc.sync.dma_start(stab[:], stab_ap[:])
            nc.sync.dma_start(stabf[:], stabf_ap[:])

            # DMA pacing: trig copies on the ACT queue gate the next issues
            # on an earlier chunk's ARRIVAL (not on compute progress).
            def trig_then(gate_tile, thunks):
                trg = trgp.tile([128, 8], bf16, tag="trg", name="trg")
                nc.scalar.copy(trg[:], gate_tile[:, 0:8])
                for th in thunks:
                    th()

            trig_then(H0[0], [
                lambda: issue_h0(2, nc.scalar),
                lambda: issue_h0(3, nc.scalar),
                lambda: issue_mst(1, 0, nc.scalar),
            ])

            # issues attached after job i's evac (scalar queue, in order)
            sched = {
                0: [lambda: issue_h0(4, nc.scalar),
                    lambda: issue_h0(5, nc.scalar),
                    lambda: issue_mst(0, 1, nc.scalar)],
                1: [lambda: issue_h0(6, nc.scalar),
                    lambda: issue_h0(7, nc.scalar)],
                2: [lambda: issue_mst(1, 1, nc.scalar)],
                3: [lambda: issue_mst(2, 0, nc.scalar)],
                4: [lambda: issue_mst(2, 1, nc.scalar)],
                5: [lambda: issue_mst(3, 0, nc.scalar)],
                6: [lambda: issue_mst(3, 1, nc.scalar)],
            }

            E2 = {}      # (r, mp) -> [128, 1024] bf16 (tiles 4*(2mp)+r | 4*(2mp+1)+r)
            Y = {}
            Z = {}
            H11 = {}
            pend11 = []  # deferred stt11 work items (r, m)

            def Ecol(r, m):
                tl = E2[(r, m >> 1)]
                h = m & 1
                return tl[:, h * 512:(h + 1) * 512]

            def emit_stt11(r, m, eng):
                t = 4 * m + r
                H11[t] = h11p.tile([128, BSH], bf16, tag="h11", name=f"H11_{t}")
                eng.scalar_tensor_tensor(
                    H11[t][:], Z[4 * (m ^ 4) + r][:], stab[:, 128 + 2 * t:128 + 2 * t + 1],
                    Z[t][:], op0=mult, op1=add,
                )
                nc.sync.dma_start(out_ap[t * 128:(t + 1) * 128, :], H11[t][:])

            for idx, (r, mp) in enumerate(_JOBS):
                # ---- phase 1: 8 matmuls into a [128, 1024] psum pair
                P2 = psp.tile([128, 1024], f32, tag="ps", name=f"p2_{r}_{mp}")
                for h, m in enumerate((2 * mp, 2 * mp + 1)):
                    for ji in range(4):
                        nc.tensor.matmul(
                            P2[:, h * 512:(h + 1) * 512],
                            mst_slice(r, m, ji),
                            H0[m][:, (r ^ ji) * 512:((r ^ ji) + 1) * 512],
                            start=(ji == 0), stop=(ji == 3),
                        )
                # ---- fused ACT evacuation to bf16
                e2 = e2p.tile([128, 1024], bf16, tag="e2", name=f"e2_{r}_{mp}")
                nc.scalar.copy(e2[:], P2[:])
                E2[(r, mp)] = e2
                for th in sched.get(idx, []):
                    th()

                # ---- stt9 on DVE (partners t, t^4 are the two e2 halves)
                for h, m in enumerate((2 * mp, 2 * mp + 1)):
                    t = 4 * m + r
                    Y[t] = yp.tile([128, BSH], bf16, tag="y", name=f"Y_{t}")
                    nc.vector.scalar_tensor_tensor(
                        Y[t][:], Ecol(r, m ^ 1), stab[:, 2 * t:2 * t + 1],
                        Ecol(r, m), op0=mult, op1=add,
                    )

                # ---- drain up to 2 deferred stt11 (DVE) + their stores
                for _ in range(2):
                    if pend11:
                        pr, pm = pend11.pop(0)
                        emit_stt11(pr, pm, nc.vector)

                # ---- stt10 on GPSIMD per completed half (partners m, m^2)
                if mp in (1, 3):
                    for m in (range(0, 4) if mp == 1 else range(4, 8)):
                        t = 4 * m + r
                        w = wkp.tile([128, BSH], bf16, tag="wk", name=f"W_{t}")
                        if m in (0, 1, 4, 5):
                            nc.vector.tensor_scalar_mul(
                                w[:], Y[4 * (m ^ 2) + r][:],
                                stabf[:, t:t + 1],
                            )
                        else:
                            nc.scalar.mul(
                                w[:], Y[4 * (m ^ 2) + r][:],
                                stabf[:, t:t + 1],
                            )
                        Z[t] = zp.tile([128, BSH], bf16, tag="z", name=f"Z_{t}")
                        nc.vector.tensor_add(Z[t][:], w[:], Y[t][:])
                if mp == 3:
                    pend11.extend((r, m) for m in range(8))

            # tail: drain remaining stt11 on DVE
            for pr, pm in pend11:
                emit_stt11(pr, pm, nc.vector)

    nc.compile()
    return nc


def kernel(x, values, idx_in, idx_out):
    global LAST_EXEC_NS
    from concourse.bass_utils import run_bass_kernel_spmd

    x = np.asarray(x, np.float32)
    assert x.shape == (BATCH, N), x.shape
    mst, stab, stabf = _host_precompute(values, idx_in, idx_out)
    xT = np.ascontiguousarray(x.T.astype(ml_dtypes.bfloat16))

    if "prog" not in _PROGRAM_CACHE:
        _PROGRAM_CACHE["prog"] = _build_program()
    nc = _PROGRAM_CACHE["prog"]

    in_maps = [
        {
            "xT": np.ascontiguousarray(xT[:, i * BSH:(i + 1) * BSH]),
            "mst": mst,
            "stab": stab,
            "stabf": stabf,
        }
        for i in range(NCORES)
    ]
    res = run_bass_kernel_spmd(nc, in_maps, core_ids=list(range(NCORES)))
    if os.environ.get("BENES_TRACE"):
        tres = run_bass_kernel_spmd(
            nc, in_maps, core_ids=list(range(NCORES)), trace=True
        )
        LAST_EXEC_NS = tres.exec_time_ns
        _PROGRAM_CACHE["profile_json"] = tres.profile_json
    out = np.empty((BATCH, N), np.float32)
    for i in range(NCORES):
        out[i * BSH:(i + 1) * BSH] = (
            np.asarray(res.results[i]["outT"]).T.astype(np.float32)
        )
    return out
